# revision 14
# baseline (speedup 1.0000x reference)
"""ChemResBlock kernel for Trainium2, 8 NeuronCores.

Math: each conv is out[a,o] = sum_{k,f} conn[a,k,f] * V[k,o,f] + bond[a,o]
with V[k,o,f] = (sum_d h[k,d] * pf[o,f,d]) * bf[o,f,0]
and bond[a,o] = sum_{f,c} bond_property[a,f,c] * bf[o,f,1+c].

Sharding: rows (output atoms) split 8 ways. Each core keeps its transposed
connectivity slice [k*f, a_local] resident in SBUF as bf16 (12.6 MB) and
reuses it across all 4 convs. Node features h are all-gathered between convs.
"""

import sys

import numpy as np
import ml_dtypes

sys.path.insert(0, "/opt/trn_rl_repo")

A = 2048
D = 64
FL = 12
NCORES = 8
AS = A // NCORES  # 256 output atoms per core
KT = A // 128  # 16 contraction k-tiles
FO = FL * D  # 768

F16 = np.float16

_CACHE = {}


def _build_program():
    import concourse.bacc as bacc
    import concourse.mybir as mybir
    import concourse.tile as tile

    f32 = mybir.dt.float32
    f16 = mybir.dt.float16

    nc = bacc.Bacc(
        "TRN2", target_bir_lowering=False, debug=False, num_devices=NCORES
    )

    # Per-core inputs (ct/xTs/bpT differ per core, rest replicated).
    ct = nc.dram_tensor("ct", [KT, 128, FL, AS], f16, kind="ExternalInput")
    xTbf = nc.dram_tensor("xTbf", [D, A], f16, kind="ExternalInput")
    xTs = nc.dram_tensor("xTs", [D, AS], f32, kind="ExternalInput")
    pfc = nc.dram_tensor("pfc", [2, D, FO], f16, kind="ExternalInput")
    bfc = nc.dram_tensor("bfc", [2, 24, D], f32, kind="ExternalInput")
    bpT = nc.dram_tensor("bpT", [24, AS], f32, kind="ExternalInput")
    outT = nc.dram_tensor("outT", [D, AS], f32, kind="ExternalOutput")

    with tile.TileContext(nc) as tc:
        with (
            tc.tile_pool(name="const", bufs=1) as cpool,
            tc.tile_pool(name="work", bufs=2) as wpool,
            tc.tile_pool(name="vps", bufs=3, space="PSUM") as vps_pool,
            tc.tile_pool(name="accps", bufs=2, space="PSUM") as acc_pool,
            tc.tile_pool(name="dram", bufs=1, space="DRAM") as dram,
        ):
            # Dummy collective up front: rings the CC doorbell while the PE
            # is busy with conv 0, absorbing the cold-start cost (~20 us) the
            # first real all-gather would otherwise pay. Gates nothing.
            warm_in = dram.tile([D, 8], f16, tag="warm_in")
            warm_out = dram.tile([NCORES * D, 8], f16, tag="warm_out", addr_space="Shared")
            nc.gpsimd.collective_compute(
                "AllGather",
                mybir.AluOpType.bypass,
                replica_groups=[list(range(NCORES))],
                ins=[warm_in.opt()],
                outs=[warm_out.opt()],
            )

            # Small constants first so they are not FIFO-queued behind the
            # 12.6 MB connectivity load. h and Pf live twice (partitions 0:64
            # and 64:128) so two k-tiles' V-steps can run in separate PE row
            # groups concurrently.
            xTbf_sb = cpool.tile([2 * D, A], f16)
            nc.sync.dma_start(
                xTbf_sb[:], xTbf[None].broadcast_to([2, D, A])
            )
            pfc_sb = cpool.tile([2 * D, 2, FO], f16)
            nc.scalar.dma_start(pfc_sb[0:D], pfc.rearrange("l d fo -> d l fo"))
            nc.scalar.dma_start(
                pfc_sb[D : 2 * D], pfc.rearrange("l d fo -> d l fo")
            )
            xTs_sb = cpool.tile([D, AS], f32)
            nc.scalar.dma_start(xTs_sb[:], xTs[:])
            bfc_sb = cpool.tile([24, 2, D], f32)
            nc.scalar.dma_start(bfc_sb[:], bfc.rearrange("l p o -> p l o"))
            bpT_sb = cpool.tile([24, AS], f32)
            nc.scalar.dma_start(bpT_sb[:], bpT[:])

            # Resident connectivity: [128, kt, f, a_local] fp16, 96 KB/partition,
            # split across both HWDGE rings (sync + scalar), two k-tiles per
            # transfer to amortize issue overhead.
            ct_sb = cpool.tile([128, KT, FL, AS], f16)
            for t in range(0, KT, 2):
                eng = nc.sync if t % 4 == 0 else nc.scalar
                eng.dma_start(
                    ct_sb[:, t : t + 2], ct[t : t + 2].rearrange("t p f a -> p t f a")
                )

            # bond_l[o, a_local] via one small fp32 matmul per layer; also
            # pre-add x for the residual convs: bx[layer, 0]=bond,
            # bx[layer, 1]=bond+xT.
            bond_sb = cpool.tile([D, 2, 2, AS], f32)
            for layer in range(2):
                bond_ps = acc_pool.tile([D, AS], f32, tag="acc")
                nc.tensor.matmul(
                    bond_ps[:], bfc_sb[:, layer], bpT_sb[:], start=True, stop=True
                )
                nc.vector.tensor_copy(bond_sb[:, layer, 0], bond_ps[:])
                nc.vector.tensor_add(bond_sb[:, layer, 1], bond_ps[:], xTs_sb[:])

            hT_cur = xTbf_sb  # [2*D, A] fp16, h duplicated on both halves

            def v_pair(hT2, layer, t):
                # V for k-tiles t (PE rows 0:64) and t+1 (rows 64:128).
                # The two halves' matmuls are interleaved j-wise so they run
                # concurrently in separate PE row groups.
                halves = [(0, t)]
                if t + 1 < KT:
                    halves.append((1, t + 1))
                v_ps = {}
                v_bf = {}
                for half, tt in halves:
                    v_ps[half] = vps_pool.tile(
                        [128, FO], f32, tag="vps", name=f"v_ps_{tt}"
                    )
                    v_bf[half] = wpool.tile(
                        [128, FO], f16, tag="vbf", bufs=4, name=f"v_bf_{tt}"
                    )
                for lo, hi in ((0, 512), (512, FO)):
                    for half, tt in halves:
                        nc.tensor.matmul(
                            v_ps[half][:, lo:hi],
                            hT2[half * D : (half + 1) * D, tt * 128 : (tt + 1) * 128],
                            pfc_sb[half * D : (half + 1) * D, layer, lo:hi],
                            start=True,
                            stop=True,
                        )
                for half, tt in halves:
                    # Split the cast across DVE (2/3) and ACT (1/3, slower
                    # engine) so the halves convert in parallel.
                    nc.vector.tensor_copy(v_bf[half][:, 0:512], v_ps[half][:, 0:512])
                    nc.scalar.copy(v_bf[half][:, 512:FO], v_ps[half][:, 512:FO])
                return [v_bf.get(0), v_bf.get(1)]

            for s in range(4):
                layer = s // 2
                second = s % 2 == 1
                # Two concurrent accumulation chains via PE column groups:
                # even f -> psum partitions 0:64, odd f -> 64:128.
                acc = acc_pool.tile([128, AS], f32, tag="acc")
                v_cur = v_pair(hT_cur, layer, 0)
                for tp in range(KT // 2):
                    v_next = v_pair(hT_cur, layer, 2 * tp + 2) if tp + 1 < KT // 2 else [None, None]
                    for half in range(2):
                        t = 2 * tp + half
                        for fp in range(FL // 2):
                            f0, f1 = 2 * fp, 2 * fp + 1
                            first = t == 0 and fp == 0
                            last = t == KT - 1 and fp == FL // 2 - 1
                            nc.tensor.matmul(
                                acc[0:D],
                                v_cur[half][:, f0 * D : (f0 + 1) * D],
                                ct_sb[:, t, f0],
                                start=first,
                                stop=last,
                            )
                            nc.tensor.matmul(
                                acc[D : 2 * D],
                                v_cur[half][:, f1 * D : (f1 + 1) * D],
                                ct_sb[:, t, f1],
                                start=first,
                                stop=last,
                            )
                    v_cur = v_next

                # Epilogue: relu(acc_even + acc_odd + bond (+ x)).
                tmp = wpool.tile([D, AS], f32, tag="ep")
                nc.vector.tensor_add(
                    tmp[:], acc[0:D], bond_sb[:, layer, 1 if second else 0]
                )
                nc.vector.tensor_add(tmp[:], tmp[:], acc[D : 2 * D])

                if s < 3:
                    # Fused relu + fp16 cast on ACT.
                    h_loc = wpool.tile([D, AS], f16, tag="hloc")
                    nc.scalar.activation(
                        h_loc[:], tmp[:], mybir.ActivationFunctionType.Relu
                    )
                    ag_in = dram.tile([D, AS], f16, tag=f"agin{s}")
                    ag_out = dram.tile(
                        [NCORES * D, AS], f16, tag=f"agout{s}", addr_space="Shared"
                    )
                    nc.sync.dma_start(ag_in[:], h_loc[:])
                    nc.gpsimd.collective_compute(
                        "AllGather",
                        mybir.AluOpType.bypass,
                        replica_groups=[list(range(NCORES))],
                        ins=[ag_in.opt()],
                        outs=[ag_out.opt()],
                    )
                    # One DMA per partition-half: view ag_out [(r d), a] as
                    # [d, r, a] and write hT [d, (r a)].
                    hT_next = wpool.tile([2 * D, A], f16, tag="hT")
                    src_ap = ag_out.rearrange("(r d) a -> d r a", d=D)
                    nc.sync.dma_start(
                        hT_next[0:D].rearrange("d (r a) -> d r a", r=NCORES), src_ap
                    )
                    nc.scalar.dma_start(
                        hT_next[D : 2 * D].rearrange("d (r a) -> d r a", r=NCORES),
                        src_ap,
                    )
                    hT_cur = hT_next
                else:
                    nc.vector.tensor_relu(tmp[:], tmp[:])
                    nc.sync.dma_start(outT[:], tmp[:])

    nc.compile()
    return nc


def _prep_inputs(x, connectivity, bond_property, pf0, bf0, pf1, bf1):
    x = np.asarray(x, np.float32)
    connectivity = np.asarray(connectivity, np.float32)
    bond_property = np.asarray(bond_property, np.float32)

    xTbf = np.ascontiguousarray(x.T).astype(F16)

    pfc = np.stack(
        [
            np.ascontiguousarray(
                (np.asarray(pf, np.float32) * np.asarray(bf, np.float32)[:, :, 0][:, :, None])
                .transpose(2, 1, 0)
                .reshape(D, FO)
            )
            for pf, bf in ((pf0, bf0), (pf1, bf1))
        ]
    ).astype(F16)

    bfc = np.stack(
        [
            np.ascontiguousarray(
                np.asarray(bf, np.float32)[:, :, 1:3].reshape(D, 24).T
            )
            for bf in (bf0, bf1)
        ]
    ).astype(np.float32)

    in_maps = []
    for c in range(NCORES):
        a0 = c * AS
        sl = connectivity[a0 : a0 + AS]  # [AS, A, FL]
        ct_c = np.ascontiguousarray(sl.transpose(1, 2, 0)).astype(F16)
        ct_c = ct_c.reshape(KT, 128, FL, AS)
        in_maps.append(
            {
                "ct": ct_c,
                "xTbf": xTbf,
                "xTs": np.ascontiguousarray(x[a0 : a0 + AS].T),
                "pfc": pfc,
                "bfc": bfc,
                "bpT": np.ascontiguousarray(
                    bond_property[a0 : a0 + AS].reshape(AS, 24).T
                ),
            }
        )
    return in_maps


def kernel(x, connectivity, bond_property, pf0, bf0, pf1, bf1, _trace=False, _trace_cores=None):
    from concourse import bass_utils

    if "nc" not in _CACHE:
        _CACHE["nc"] = _build_program()
    nc = _CACHE["nc"]

    in_maps = _prep_inputs(x, connectivity, bond_property, pf0, bf0, pf1, bf1)
    res = bass_utils.run_bass_kernel_spmd(
        nc, in_maps, core_ids=list(range(NCORES)), trace=_trace,
        trace_cores=_trace_cores,
    )
    _CACHE["last_result"] = res
    full_T = np.concatenate([res.results[c]["outT"] for c in range(NCORES)], axis=1)
    return np.ascontiguousarray(full_T.T)


# revision 15
# speedup vs baseline: 1.0699x; 1.0699x over previous
"""ChemResBlock kernel for Trainium2, 8 NeuronCores.

Math: each conv is out[a,o] = sum_{k,f} conn[a,k,f] * V[k,o,f] + bond[a,o]
with V[k,o,f] = (sum_d h[k,d] * pf[o,f,d]) * bf[o,f,0]
and bond[a,o] = sum_{f,c} bond_property[a,f,c] * bf[o,f,1+c].

Sharding: rows (output atoms) split 8 ways. Each core keeps its transposed
connectivity slice [k*f, a_local] resident in SBUF as bf16 (12.6 MB) and
reuses it across all 4 convs. Node features h are all-gathered between convs.
"""

import sys

import numpy as np
import ml_dtypes

sys.path.insert(0, "/opt/trn_rl_repo")

A = 2048
D = 64
FL = 12
NCORES = 8
AS = A // NCORES  # 256 output atoms per core
KT = A // 128  # 16 contraction k-tiles
FO = FL * D  # 768

F16 = np.float16

_CACHE = {}


def _build_program():
    import concourse.bacc as bacc
    import concourse.mybir as mybir
    import concourse.tile as tile

    f32 = mybir.dt.float32
    f16 = mybir.dt.float16

    nc = bacc.Bacc(
        "TRN2", target_bir_lowering=False, debug=False, num_devices=NCORES
    )

    # Per-core inputs (ct/xTs/bpT differ per core, rest replicated).
    ct = nc.dram_tensor("ct", [KT, 128, FL, AS], f16, kind="ExternalInput")
    xTbf = nc.dram_tensor("xTbf", [D, A], f16, kind="ExternalInput")
    xTs = nc.dram_tensor("xTs", [D, AS], f32, kind="ExternalInput")
    pfc = nc.dram_tensor("pfc", [2, D, FO], f16, kind="ExternalInput")
    bfc = nc.dram_tensor("bfc", [2, 24, D], f32, kind="ExternalInput")
    bpT = nc.dram_tensor("bpT", [24, AS], f32, kind="ExternalInput")
    outT = nc.dram_tensor("outT", [D, AS], f32, kind="ExternalOutput")

    with tile.TileContext(nc) as tc:
        with (
            tc.tile_pool(name="const", bufs=1) as cpool,
            tc.tile_pool(name="work", bufs=2) as wpool,
            tc.tile_pool(name="vps", bufs=3, space="PSUM") as vps_pool,
            tc.tile_pool(name="accps", bufs=2, space="PSUM") as acc_pool,
            tc.tile_pool(name="dram", bufs=1, space="DRAM") as dram,
        ):
            # Dummy collective up front: rings the CC doorbell while the PE
            # is busy with conv 0, absorbing the cold-start cost (~20 us) the
            # first real all-gather would otherwise pay. Gates nothing.
            warm_in = dram.tile([D, 8], f16, tag="warm_in")
            warm_out = dram.tile([NCORES * D, 8], f16, tag="warm_out", addr_space="Shared")
            nc.gpsimd.collective_compute(
                "AllGather",
                mybir.AluOpType.bypass,
                replica_groups=[list(range(NCORES))],
                ins=[warm_in.opt()],
                outs=[warm_out.opt()],
            )

            # Small constants first so they are not FIFO-queued behind the
            # 12.6 MB connectivity load. h and Pf live twice (partitions 0:64
            # and 64:128) so two k-tiles' V-steps can run in separate PE row
            # groups concurrently.
            xTbf_sb = cpool.tile([2 * D, A], f16)
            nc.sync.dma_start(xTbf_sb[0:D], xTbf[:])
            nc.sync.dma_start(xTbf_sb[D : 2 * D], xTbf[:])
            pfc_sb = cpool.tile([2 * D, 2, FO], f16)
            nc.scalar.dma_start(pfc_sb[0:D], pfc.rearrange("l d fo -> d l fo"))
            nc.scalar.dma_start(
                pfc_sb[D : 2 * D], pfc.rearrange("l d fo -> d l fo")
            )
            xTs_sb = cpool.tile([D, AS], f32)
            nc.scalar.dma_start(xTs_sb[:], xTs[:])
            bfc_sb = cpool.tile([24, 2, D], f32)
            nc.scalar.dma_start(bfc_sb[:], bfc.rearrange("l p o -> p l o"))
            bpT_sb = cpool.tile([24, AS], f32)
            nc.scalar.dma_start(bpT_sb[:], bpT[:])

            # Resident connectivity: [128, kt, f, a_local] fp16, 96 KB/partition,
            # split across both HWDGE rings (sync + scalar), two k-tiles per
            # transfer to amortize issue overhead.
            ct_sb = cpool.tile([128, KT, FL, AS], f16)
            for t in range(0, KT, 2):
                eng = nc.sync if t % 4 == 0 else nc.scalar
                eng.dma_start(
                    ct_sb[:, t : t + 2], ct[t : t + 2].rearrange("t p f a -> p t f a")
                )

            # bond_l[o, a_local] via one small fp32 matmul per layer; also
            # pre-add x for the residual convs: bx[layer, 0]=bond,
            # bx[layer, 1]=bond+xT.
            bond_sb = cpool.tile([D, 2, 2, AS], f32)
            for layer in range(2):
                bond_ps = acc_pool.tile([D, AS], f32, tag="acc")
                nc.tensor.matmul(
                    bond_ps[:], bfc_sb[:, layer], bpT_sb[:], start=True, stop=True
                )
                nc.vector.tensor_copy(bond_sb[:, layer, 0], bond_ps[:])
                nc.vector.tensor_add(bond_sb[:, layer, 1], bond_ps[:], xTs_sb[:])

            hT_cur = xTbf_sb  # [2*D, A] fp16, h duplicated on both halves

            def v_pair(hT2, layer, t):
                # V for k-tiles t (PE rows 0:64) and t+1 (rows 64:128).
                # The two halves' matmuls are interleaved j-wise so they run
                # concurrently in separate PE row groups.
                halves = [(0, t)]
                if t + 1 < KT:
                    halves.append((1, t + 1))
                v_ps = {}
                v_bf = {}
                for half, tt in halves:
                    v_ps[half] = vps_pool.tile(
                        [128, FO], f32, tag="vps", name=f"v_ps_{tt}"
                    )
                    v_bf[half] = wpool.tile(
                        [128, FO], f16, tag="vbf", bufs=4, name=f"v_bf_{tt}"
                    )
                for lo, hi in ((0, 512), (512, FO)):
                    for half, tt in halves:
                        nc.tensor.matmul(
                            v_ps[half][:, lo:hi],
                            hT2[half * D : (half + 1) * D, tt * 128 : (tt + 1) * 128],
                            pfc_sb[half * D : (half + 1) * D, layer, lo:hi],
                            start=True,
                            stop=True,
                        )
                for half, tt in halves:
                    # Split the cast across DVE (2/3) and ACT (1/3, slower
                    # engine) so the halves convert in parallel.
                    nc.vector.tensor_copy(v_bf[half][:, 0:512], v_ps[half][:, 0:512])
                    nc.scalar.copy(v_bf[half][:, 512:FO], v_ps[half][:, 512:FO])
                return [v_bf.get(0), v_bf.get(1)]

            for s in range(4):
                layer = s // 2
                second = s % 2 == 1
                # Two concurrent accumulation chains via PE column groups:
                # even f -> psum partitions 0:64, odd f -> 64:128.
                acc = acc_pool.tile([128, AS], f32, tag="acc")
                v_cur = v_pair(hT_cur, layer, 0)
                for tp in range(KT // 2):
                    v_next = v_pair(hT_cur, layer, 2 * tp + 2) if tp + 1 < KT // 2 else [None, None]
                    for half in range(2):
                        t = 2 * tp + half
                        for fp in range(FL // 2):
                            f0, f1 = 2 * fp, 2 * fp + 1
                            first = t == 0 and fp == 0
                            last = t == KT - 1 and fp == FL // 2 - 1
                            nc.tensor.matmul(
                                acc[0:D],
                                v_cur[half][:, f0 * D : (f0 + 1) * D],
                                ct_sb[:, t, f0],
                                start=first,
                                stop=last,
                            )
                            nc.tensor.matmul(
                                acc[D : 2 * D],
                                v_cur[half][:, f1 * D : (f1 + 1) * D],
                                ct_sb[:, t, f1],
                                start=first,
                                stop=last,
                            )
                    v_cur = v_next

                # Epilogue: relu(acc_even + acc_odd + bond (+ x)).
                tmp = wpool.tile([D, AS], f32, tag="ep")
                nc.vector.tensor_add(
                    tmp[:], acc[0:D], bond_sb[:, layer, 1 if second else 0]
                )
                nc.vector.tensor_add(tmp[:], tmp[:], acc[D : 2 * D])

                if s < 3:
                    # Fused relu + fp16 cast on ACT.
                    h_loc = wpool.tile([D, AS], f16, tag="hloc")
                    nc.scalar.activation(
                        h_loc[:], tmp[:], mybir.ActivationFunctionType.Relu
                    )
                    ag_in = dram.tile([D, AS], f16, tag=f"agin{s}")
                    ag_out = dram.tile(
                        [NCORES * D, AS], f16, tag=f"agout{s}", addr_space="Shared"
                    )
                    nc.sync.dma_start(ag_in[:], h_loc[:])
                    nc.gpsimd.collective_compute(
                        "AllGather",
                        mybir.AluOpType.bypass,
                        replica_groups=[list(range(NCORES))],
                        ins=[ag_in.opt()],
                        outs=[ag_out.opt()],
                    )
                    # One DMA per partition-half: view ag_out [(r d), a] as
                    # [d, r, a] and write hT [d, (r a)].
                    hT_next = wpool.tile([2 * D, A], f16, tag="hT")
                    src_ap = ag_out.rearrange("(r d) a -> d r a", d=D)
                    nc.sync.dma_start(
                        hT_next[0:D].rearrange("d (r a) -> d r a", r=NCORES), src_ap
                    )
                    nc.scalar.dma_start(
                        hT_next[D : 2 * D].rearrange("d (r a) -> d r a", r=NCORES),
                        src_ap,
                    )
                    hT_cur = hT_next
                else:
                    nc.vector.tensor_relu(tmp[:], tmp[:])
                    nc.sync.dma_start(outT[:], tmp[:])

    nc.compile()
    return nc


def _prep_inputs(x, connectivity, bond_property, pf0, bf0, pf1, bf1):
    x = np.asarray(x, np.float32)
    connectivity = np.asarray(connectivity, np.float32)
    bond_property = np.asarray(bond_property, np.float32)

    xTbf = np.ascontiguousarray(x.T).astype(F16)

    pfc = np.stack(
        [
            np.ascontiguousarray(
                (np.asarray(pf, np.float32) * np.asarray(bf, np.float32)[:, :, 0][:, :, None])
                .transpose(2, 1, 0)
                .reshape(D, FO)
            )
            for pf, bf in ((pf0, bf0), (pf1, bf1))
        ]
    ).astype(F16)

    bfc = np.stack(
        [
            np.ascontiguousarray(
                np.asarray(bf, np.float32)[:, :, 1:3].reshape(D, 24).T
            )
            for bf in (bf0, bf1)
        ]
    ).astype(np.float32)

    in_maps = []
    for c in range(NCORES):
        a0 = c * AS
        sl = connectivity[a0 : a0 + AS]  # [AS, A, FL]
        ct_c = np.ascontiguousarray(sl.transpose(1, 2, 0)).astype(F16)
        ct_c = ct_c.reshape(KT, 128, FL, AS)
        in_maps.append(
            {
                "ct": ct_c,
                "xTbf": xTbf,
                "xTs": np.ascontiguousarray(x[a0 : a0 + AS].T),
                "pfc": pfc,
                "bfc": bfc,
                "bpT": np.ascontiguousarray(
                    bond_property[a0 : a0 + AS].reshape(AS, 24).T
                ),
            }
        )
    return in_maps


def kernel(x, connectivity, bond_property, pf0, bf0, pf1, bf1, _trace=False, _trace_cores=None):
    from concourse import bass_utils

    if "nc" not in _CACHE:
        _CACHE["nc"] = _build_program()
    nc = _CACHE["nc"]

    in_maps = _prep_inputs(x, connectivity, bond_property, pf0, bf0, pf1, bf1)
    res = bass_utils.run_bass_kernel_spmd(
        nc, in_maps, core_ids=list(range(NCORES)), trace=_trace,
        trace_cores=_trace_cores,
    )
    _CACHE["last_result"] = res
    full_T = np.concatenate([res.results[c]["outT"] for c in range(NCORES)], axis=1)
    return np.ascontiguousarray(full_T.T)
